# revision 15
# baseline (speedup 1.0000x reference)
"""CenterLoss on Trainium2 (raw Bass, SPMD over 8 NeuronCores).

Computes mean_i ||x_i - centers[label_i]||^2 (the reference clamps each
distance to [1e-12, 1e12], which never binds for this data regime).

Sharding (vocab/class-parallel, per the hint):
  - centers [100000, 512] is split row-wise into 8 shards of 12500 rows.
  - x [256, 512] and the labels are replicated to all cores.
  - Host-side sharding prep: per-core local labels = label - shard_base,
    with out-of-shard labels replaced by a huge sentinel that the
    gather's bounds check drops; a per-core f32 weight vector holds
    1/256 for in-shard rows and 0 otherwise.  Each core produces
    sum(weight_i * dist_i) — its partial of the final mean — and the
    host sums the 8 partial scalars (the unshard step).

Device program per core (identical SPMD image, different data):
  layout: batch row (p*2 + t) lives at partition p, column t (keeps
  every DMA innermost-contiguous; row order is irrelevant to the sum).
    lt [128, 2]     i32   <- local labels   (HWDGE, issued first)
    mt [128, 2]     f32   <- weights        (HWDGE)
    xt [128, 2*512] f32   <- x              (HWDGE, 512 KB)
    gt [128, 2*512] f32   memset 0 (DVE), then ONE indirect SWDGE
                          gather of all 256 rows (offset AP [128,2]):
                          row (p,t) <- centers_shard[lt[p,t]]
  DVE:  gt = xt - gt
  ACT:  per column t: Square activation with accum_out -> rs[:, t]
        (the activation table is prefetched by a dummy square at kernel
        start, overlapping the DMA waits)
  PE :  acc[1,1] += mt[:, t]^T @ rs[:, t]  (t = 0,1; PSUM accumulate).
        Out-of-shard rows have weight 0, so whatever the bounds-checked
        gather leaves there (0 from the memset, or x after the sub)
        contributes nothing; the memset keeps every value finite.
  DVE:  res <- acc (PSUM -> SBUF); HWDGE stores the [1,1] scalar.

Raw Bass (not Tile) because this container's walrus build accepts only
one folded sync-wait per instruction ("Too many sync wait commands") and
rejects the fused DVE tensor_tensor_reduce encoding ("ISA wrong length");
standalone wait_ge instructions and BIR-native ops sidestep both.
"""

import numpy as np

import concourse.bass as bass
from concourse import mybir
from concourse.bass_utils import run_bass_kernel_spmd

NUM_CLASSES = 100000
FEAT = 512
BATCH = 256
N_CORES = 8
ROWS = NUM_CLASSES // N_CORES  # 12500 center rows per core
P = 128
NT = BATCH // P  # 2 columns per partition
OOB_SENTINEL = 2_000_000_000  # > bounds_check, still valid int32

_cache: dict = {}

# test.py reads this after calling kernel() for exec_time_ns / trace.
LAST_RESULTS = None


def _build() -> bass.Bass:
    nc = bass.Bass(enable_partition_id=False)
    x = nc.dram_tensor("x", [BATCH, FEAT], mybir.dt.float32, kind="ExternalInput")
    lab = nc.dram_tensor("lab", [BATCH], mybir.dt.int32, kind="ExternalInput")
    msk = nc.dram_tensor("msk", [BATCH], mybir.dt.float32, kind="ExternalInput")
    cen = nc.dram_tensor("cen", [ROWS, FEAT], mybir.dt.float32, kind="ExternalInput")
    out = nc.dram_tensor("out", [1, 1], mybir.dt.float32, kind="ExternalOutput")

    # batch row (p*NT + t) -> partition p, column t
    x_v = x.rearrange("(p t) d -> p t d", t=NT)
    lab_v = lab.rearrange("(p t) -> p t", t=NT)
    msk_v = msk.rearrange("(p t) -> p t", t=NT)

    with (
        nc.sbuf_tensor([P, NT * FEAT], mybir.dt.float32) as xt,
        nc.sbuf_tensor([P, NT * FEAT], mybir.dt.float32) as gt,
        nc.sbuf_tensor([P, NT * FEAT], mybir.dt.float32) as sq,
        nc.sbuf_tensor([P, NT], mybir.dt.int32) as lt,
        nc.sbuf_tensor([P, NT], mybir.dt.float32) as mt,
        nc.sbuf_tensor([P, NT], mybir.dt.float32) as rs,
        nc.sbuf_tensor([1, 1], mybir.dt.float32) as res,
        nc.psum_tensor([1, 1], mybir.dt.float32) as acc,
        nc.semaphore() as s_x,    # xt load done (+16)
        nc.semaphore() as s_l,    # lt load done (+16)
        nc.semaphore() as s_m,    # mt load done (+16)
        nc.semaphore() as s_ms,   # gt memset done (+1)
        nc.semaphore() as s_g,    # gather done (+16)
        nc.semaphore() as s_v,    # DVE sub done (+1)
        nc.semaphore() as s_a,    # ACT squares done (+1 each)
        nc.semaphore() as s_mm,   # PE matmuls done (+1)
        nc.semaphore() as s_res,  # res copy done (+1)
        nc.semaphore() as s_out,  # final store done (+16)
        nc.Block() as block,
    ):
        gt3 = gt[:].rearrange("p (t d) -> p t d", t=NT)
        sq3 = sq[:].rearrange("p (t d) -> p t d", t=NT)
        xt3 = xt[:].rearrange("p (t d) -> p t d", t=NT)

        @block.sync
        def _(sync: bass.BassEngine):
            sync.dma_start(out=lt[:], in_=lab_v).then_inc(s_l, 16)
            sync.dma_start(out=mt[:], in_=msk_v).then_inc(s_m, 16)
            sync.dma_start(out=xt3, in_=x_v).then_inc(s_x, 16)
            sync.wait_ge(s_res, 1)
            sync.dma_start(out=out[:], in_=res[:]).then_inc(s_out, 16)
            sync.wait_ge(s_out, 16)

        @block.gpsimd
        def _(gpsimd: bass.BassEngine):
            gpsimd.wait_ge(s_ms, 1)
            gpsimd.wait_ge(s_l, 16)
            gpsimd.indirect_dma_start(
                out=gt3,
                out_offset=None,
                in_=cen[:],
                in_offset=bass.IndirectOffsetOnAxis(ap=lt[:, :], axis=0),
                bounds_check=ROWS - 1,
                oob_is_err=False,
            ).then_inc(s_g, 16)

        @block.vector
        def _(vector: bass.BassEngine):
            vector.memset(gt[:], 0.0).then_inc(s_ms, 1)
            vector.wait_ge(s_x, 16)
            vector.wait_ge(s_g, 16)
            vector.tensor_sub(out=gt[:], in0=xt[:], in1=gt[:]).then_inc(s_v, 1)
            vector.wait_ge(s_mm, 1)
            vector.tensor_copy(out=res[:], in_=acc[:]).then_inc(s_res, 1)

        @block.scalar
        def _(scalar: bass.BassEngine):
            # Dummy square: prefetches the ACT function table while the
            # DMAs are still in flight (first ACTIVATE triggers the load).
            scalar.square(out=res[:], in_=res[:])
            scalar.wait_ge(s_v, 1)
            for t in range(NT):
                scalar.activation(
                    out=sq3[:, t, :],
                    in_=gt3[:, t, :],
                    func=mybir.ActivationFunctionType.Square,
                    accum_out=rs[:, t : t + 1],
                ).then_inc(s_a, 1)

        @block.tensor
        def _(tensor: bass.BassEngine):
            tensor.wait_ge(s_m, 16)
            tensor.wait_ge(s_a, 2)
            for t in range(NT):
                mm = tensor.matmul(
                    out=acc[:],
                    lhsT=mt[:, t : t + 1],
                    rhs=rs[:, t : t + 1],
                    start=(t == 0),
                    stop=(t == NT - 1),
                )
            mm.then_inc(s_mm, 1)

    return nc


def kernel(x: np.ndarray, label: np.ndarray, centers: np.ndarray) -> np.ndarray:
    global LAST_RESULTS
    x = np.ascontiguousarray(np.asarray(x, dtype=np.float32))
    centers = np.ascontiguousarray(np.asarray(centers, dtype=np.float32))
    lbl = np.asarray(label).astype(np.int64).ravel()
    assert x.shape == (BATCH, FEAT), x.shape
    assert centers.shape == (NUM_CLASSES, FEAT), centers.shape
    assert lbl.shape == (BATCH,), lbl.shape

    in_maps = []
    for i in range(N_CORES):
        loc = lbl - i * ROWS
        valid = (loc >= 0) & (loc < ROWS)
        loc32 = np.where(valid, loc, OOB_SENTINEL).astype(np.int32)
        wt = valid.astype(np.float32) / np.float32(BATCH)
        in_maps.append(
            {
                "x": x,
                "lab": loc32,
                "msk": wt,
                "cen": centers[i * ROWS : (i + 1) * ROWS],
            }
        )

    if "nc" not in _cache:
        _cache["nc"] = _build()
    res = run_bass_kernel_spmd(_cache["nc"], in_maps, core_ids=list(range(N_CORES)))
    LAST_RESULTS = res

    total = np.float64(0.0)
    for r in res.results:
        total += np.float64(r["out"][0, 0])
    return np.float32(total)


# revision 16
# speedup vs baseline: 1.4270x; 1.4270x over previous
"""CenterLoss on Trainium2 (raw Bass, SPMD over 8 NeuronCores).

Computes mean_i ||x_i - centers[label_i]||^2 (the reference clamps each
distance to [1e-12, 1e12], which never binds for this data regime).

Sharding (vocab/class-parallel, per the hint):
  - centers [100000, 512] is split row-wise into 8 shards of 12500 rows.
  - x [256, 512] and the labels are replicated to all cores.
  - Host-side sharding prep: per-core local labels = label - shard_base,
    with out-of-shard labels replaced by a huge sentinel that the
    gather's bounds check drops; a per-core f32 weight vector holds
    1/256 for in-shard rows and 0 otherwise.  Each core produces
    sum(weight_i * dist_i) — its partial of the final mean — and the
    host sums the 8 partial scalars (the unshard step).

Device program per core (identical SPMD image, different data):
  layout: batch row (p*2 + t) lives at partition p, column t (keeps
  every DMA innermost-contiguous; row order is irrelevant to the sum).
    lt [128, 2]     i32   <- local labels   (HWDGE, issued first)
    mt [128, 2]     f32   <- weights        (HWDGE)
    xt [128, 2*512] f32   <- x              (HWDGE, 512 KB)
    gt [128, 2*512] f32   memset 0 (DVE), then ONE indirect SWDGE
                          gather of all 256 rows (offset AP [128,2]):
                          row (p,t) <- centers_shard[lt[p,t]]
  DVE:  gt = xt - gt
  ACT:  per column t: Square activation with accum_out -> rs[:, t]
        (the activation table is prefetched by a dummy square at kernel
        start, overlapping the DMA waits)
  PE :  acc[1,1] += mt[:, t]^T @ rs[:, t]  (t = 0,1; PSUM accumulate).
        Out-of-shard rows have weight 0, so whatever the bounds-checked
        gather leaves there (0 from the memset, or x after the sub)
        contributes nothing; the memset keeps every value finite.
  DVE:  res <- acc (PSUM -> SBUF); HWDGE stores the [1,1] scalar.

Raw Bass (not Tile) because this container's walrus build accepts only
one folded sync-wait per instruction ("Too many sync wait commands") and
rejects the fused DVE tensor_tensor_reduce encoding ("ISA wrong length");
standalone wait_ge instructions and BIR-native ops sidestep both.
"""

import numpy as np

import concourse.bass as bass
from concourse import mybir
from concourse.bass_utils import run_bass_kernel_spmd

NUM_CLASSES = 100000
FEAT = 512
BATCH = 256
N_CORES = 8
ROWS = NUM_CLASSES // N_CORES  # 12500 center rows per core
P = 128
NT = BATCH // P  # 2 columns per partition
OOB_SENTINEL = 2_000_000_000  # > bounds_check, still valid int32

_cache: dict = {}

# test.py reads this after calling kernel() for exec_time_ns / trace.
LAST_RESULTS = None


def _build() -> bass.Bass:
    nc = bass.Bass(enable_partition_id=False)
    x = nc.dram_tensor("x", [BATCH, FEAT], mybir.dt.float32, kind="ExternalInput")
    lab = nc.dram_tensor("lab", [BATCH], mybir.dt.int32, kind="ExternalInput")
    msk = nc.dram_tensor("msk", [BATCH], mybir.dt.float32, kind="ExternalInput")
    cen = nc.dram_tensor("cen", [ROWS, FEAT], mybir.dt.float32, kind="ExternalInput")
    out = nc.dram_tensor("out", [1, 1], mybir.dt.float32, kind="ExternalOutput")

    # batch row (p*NT + t) -> partition p, column t
    x_v = x.rearrange("(p t) d -> p t d", t=NT)
    lab_v = lab.rearrange("(p t) -> p t", t=NT)
    msk_v = msk.rearrange("(p t) -> p t", t=NT)

    with (
        nc.sbuf_tensor([P, NT * FEAT], mybir.dt.float32) as xt,
        nc.sbuf_tensor([P, NT * FEAT], mybir.dt.float32) as gt,
        nc.sbuf_tensor([P, NT * FEAT], mybir.dt.float32) as sq,
        nc.sbuf_tensor([P, NT], mybir.dt.int32) as lt,
        nc.sbuf_tensor([P, NT], mybir.dt.float32) as mt,
        nc.sbuf_tensor([P, NT], mybir.dt.float32) as rs,
        nc.sbuf_tensor([1, 1], mybir.dt.float32) as res,
        nc.psum_tensor([1, 1], mybir.dt.float32) as acc,
        nc.semaphore() as s_x,    # xt load done (+16)
        nc.semaphore() as s_l,    # lt load done (+16)
        nc.semaphore() as s_m,    # mt load done (+16)
        nc.semaphore() as s_ms,   # gt memset done (+1)
        nc.semaphore() as s_g,    # gather done (+16)
        nc.semaphore() as s_v,    # DVE sub done (+1)
        nc.semaphore() as s_a,    # ACT squares done (+1 each)
        nc.semaphore() as s_mm,   # PE matmuls done (+1)
        nc.semaphore() as s_res,  # res copy done (+1)
        nc.semaphore() as s_out,  # final store done (+16)
        nc.Block() as block,
    ):
        gt3 = gt[:].rearrange("p (t d) -> p t d", t=NT)
        sq3 = sq[:].rearrange("p (t d) -> p t d", t=NT)
        xt3 = xt[:].rearrange("p (t d) -> p t d", t=NT)

        @block.sync
        def _(sync: bass.BassEngine):
            sync.dma_start(out=lt[:], in_=lab_v).then_inc(s_l, 16)
            sync.dma_start(out=mt[:], in_=msk_v).then_inc(s_m, 16)
            sync.dma_start(out=xt3, in_=x_v).then_inc(s_x, 16)
            sync.wait_ge(s_res, 1)
            sync.dma_start(out=out[:], in_=res[:]).then_inc(s_out, 16)
            sync.wait_ge(s_out, 16)

        @block.gpsimd
        def _(gpsimd: bass.BassEngine):
            gpsimd.wait_ge(s_ms, 1)
            gpsimd.wait_ge(s_l, 16)
            # Two 128-index gathers: a single 256-index (offset AP
            # [128,2]) gather measured ~3x slower end-to-end.  Same
            # SWDGE queue -> gather t=0's semaphore fires first, letting
            # the t=0 subtract overlap the t=1 gather's transfer.
            for t in range(NT):
                gpsimd.indirect_dma_start(
                    out=gt3[:, t, :],
                    out_offset=None,
                    in_=cen[:],
                    in_offset=bass.IndirectOffsetOnAxis(ap=lt[:, t : t + 1], axis=0),
                    bounds_check=ROWS - 1,
                    oob_is_err=False,
                ).then_inc(s_g, 16)

        @block.vector
        def _(vector: bass.BassEngine):
            vector.memset(gt[:], 0.0).then_inc(s_ms, 1)
            vector.wait_ge(s_x, 16)
            for t in range(NT):
                vector.wait_ge(s_g, 16 * (t + 1))
                vector.tensor_sub(
                    out=gt3[:, t, :], in0=xt3[:, t, :], in1=gt3[:, t, :]
                ).then_inc(s_v, 1)
            vector.wait_ge(s_mm, 1)
            vector.tensor_copy(out=res[:], in_=acc[:]).then_inc(s_res, 1)

        @block.scalar
        def _(scalar: bass.BassEngine):
            # Dummy square: prefetches the ACT function table while the
            # DMAs are still in flight (first ACTIVATE triggers the load).
            scalar.square(out=res[:], in_=res[:])
            for t in range(NT):
                scalar.wait_ge(s_v, t + 1)
                scalar.activation(
                    out=sq3[:, t, :],
                    in_=gt3[:, t, :],
                    func=mybir.ActivationFunctionType.Square,
                    accum_out=rs[:, t : t + 1],
                ).then_inc(s_a, 1)

        @block.tensor
        def _(tensor: bass.BassEngine):
            tensor.wait_ge(s_m, 16)
            tensor.wait_ge(s_a, 2)
            for t in range(NT):
                mm = tensor.matmul(
                    out=acc[:],
                    lhsT=mt[:, t : t + 1],
                    rhs=rs[:, t : t + 1],
                    start=(t == 0),
                    stop=(t == NT - 1),
                )
            mm.then_inc(s_mm, 1)

    return nc


def kernel(x: np.ndarray, label: np.ndarray, centers: np.ndarray) -> np.ndarray:
    global LAST_RESULTS
    x = np.ascontiguousarray(np.asarray(x, dtype=np.float32))
    centers = np.ascontiguousarray(np.asarray(centers, dtype=np.float32))
    lbl = np.asarray(label).astype(np.int64).ravel()
    assert x.shape == (BATCH, FEAT), x.shape
    assert centers.shape == (NUM_CLASSES, FEAT), centers.shape
    assert lbl.shape == (BATCH,), lbl.shape

    in_maps = []
    for i in range(N_CORES):
        loc = lbl - i * ROWS
        valid = (loc >= 0) & (loc < ROWS)
        loc32 = np.where(valid, loc, OOB_SENTINEL).astype(np.int32)
        wt = valid.astype(np.float32) / np.float32(BATCH)
        in_maps.append(
            {
                "x": x,
                "lab": loc32,
                "msk": wt,
                "cen": centers[i * ROWS : (i + 1) * ROWS],
            }
        )

    if "nc" not in _cache:
        _cache["nc"] = _build()
    res = run_bass_kernel_spmd(_cache["nc"], in_maps, core_ids=list(range(N_CORES)))
    LAST_RESULTS = res

    total = np.float64(0.0)
    for r in res.results:
        total += np.float64(r["out"][0, 0])
    return np.float32(total)
